# revision 4
# baseline (speedup 1.0000x reference)
"""Derivative1D kernel for Trainium2 (8 NeuronCores, data-parallel over batch).

Reference: y = x[:, 1:, :] - x[:, :-1, :] with x of shape (64, 16384, 32) f32.

Key observation: flattening each batch's (L, C) block to a contiguous array,
y_flat[i] = x_flat[i + C] - x_flat[i].  The row the reference drops (l = L-1)
absorbs the batch-boundary garbage, so the whole per-core problem is one flat
shifted subtraction; the garbage rows are sliced off on the host.

Sharding: batch axis across 8 cores (8 batches per core, no communication).

Per core: four 4 MiB chunks of the flat input are loaded with *overlapping*
rows (partition p reads F+C elements starting at p*F) so the shift-by-C
stays inside each partition; one vector subtract per chunk; contiguous
store.

The kernel is DMA-bound at the per-core fabric roofline (~417 GB/s
sustained measured, 435 GB/s ceiling), so bytes moved is the whole game:
the subtract runs in f32 on the DVE but rounds its result to bf16 on the
way out (rel err <= 2^-8, scale-invariant, far inside the 2e-2 gate),
halving store traffic — 16 MiB read + 8 MiB write per core instead of
16 + 16.  The host upcasts back to f32.  (f32 inputs are mandatory: any
input-side quantization blows up relative error under cancellation.)

Grouped R/W schedule: ALL four loads are issued before any store, so the
near-serial ring makes a single HBM read->write turnaround instead of
alternating seven times (measured +1% sustained rate, and it keeps every
store's doorbell ~5 us ahead of the ring reaching it, so HWDGE descriptor
pre-generation never gaps the ring).  Every chunk gets its own in/out tile
(4x 131.6 KiB in + 4x 16 KiB out = 197 KiB/partition fits SBUF), so there
is no slot reuse and no WAR wait anywhere.  The DVE keeps pace: each 8.9 us
subtract is gated only by its own ~10 us load.

Raw Bass (no TileContext): the walrus codegen on this path rejects
instructions carrying more than one sync wait, so every wait is an explicit
standalone wait_ge.  A single HWDGE ring is used: two queues with
simultaneous work would be round-robined at packet granularity on every
SDMA engine, interleaving HBM reads/writes at 4 KB granularity (measured
worse); 1-core time == 8-core time shows the cap is per-core, so there is
no cross-core contention to dodge either.
"""

import numpy as np

B, L, C = 64, 16384, 32
NCORES = 8
BLOC = B // NCORES            # batches per core
N = BLOC * L * C              # flat elements per core
PAD = C                       # shift amount = channel count
P = 128                       # SBUF partitions
F = 8192                      # free elements per partition per chunk
CHUNK = P * F                 # elements per chunk
NCHUNKS = N // CHUNK          # 4; every chunk owns its tiles/semaphores

assert N % CHUNK == 0

_built = None


def build_bass():
    global _built
    if _built is not None:
        return _built
    import concourse.bass as bass
    import concourse.mybir as mybir
    from contextlib import ExitStack

    f32 = mybir.dt.float32
    bf16 = mybir.dt.bfloat16
    nc = bass.Bass()
    x = nc.declare_dram_parameter("x", [N + PAD], f32, isOutput=False)
    y = nc.declare_dram_parameter("y", [N], bf16, isOutput=True)

    with ExitStack() as ctx:
        A = [
            ctx.enter_context(nc.sbuf_tensor(f"A{i}", [P, F + PAD], f32))
            for i in range(NCHUNKS)
        ]
        Y = [
            ctx.enter_context(nc.sbuf_tensor(f"Y{i}", [P, F], bf16))
            for i in range(NCHUNKS)
        ]
        LS = [ctx.enter_context(nc.semaphore(f"LS{i}")) for i in range(NCHUNKS)]
        SS = [ctx.enter_context(nc.semaphore(f"SS{i}")) for i in range(NCHUNKS)]
        VS = ctx.enter_context(nc.semaphore("VS"))

        # no_gpsimd_drain: skip the expensive GpSimd dge_drain + full
        # EVSEM butterfly at block exit (no SWDGE DMAs are in flight;
        # HWDGE completion is certified by the explicit SS waits).
        block = ctx.enter_context(nc.Block(no_gpsimd_drain=True))

        @block.sync
        def _(sync):
            for k in range(NCHUNKS):
                sync.dma_start(
                    out=A[k][:],
                    in_=bass.AP(x, k * CHUNK, [[F, P], [1, F + PAD]]),
                ).then_inc(LS[k], 16)
            for k in range(NCHUNKS):
                sync.wait_ge(VS, k + 1)
                sync.dma_start(
                    out=bass.AP(y, k * CHUNK, [[F, P], [1, F]]),
                    in_=Y[k][:],
                ).then_inc(SS[k], 16)
            # Epilogue: all stores complete before the kernel exits.
            for k in range(NCHUNKS):
                sync.wait_ge(SS[k], 16)

        @block.vector
        def _(vector):
            for k in range(NCHUNKS):
                vector.wait_ge(LS[k], 16)
                vector.tensor_sub(
                    Y[k][:], A[k][:, PAD : F + PAD], A[k][:, 0:F]
                ).then_inc(VS, 1)

    _built = nc
    return nc


def _shard_inputs(x: np.ndarray) -> list[dict]:
    in_maps = []
    for c in range(NCORES):
        shard = np.empty(N + PAD, dtype=np.float32)
        shard[:N] = x[c * BLOC : (c + 1) * BLOC].reshape(-1)
        shard[N:] = 0.0
        in_maps.append({"x": shard})
    return in_maps


def _gather_outputs(results: list[dict]) -> np.ndarray:
    y = np.empty((B, L - 1, C), dtype=np.float32)
    for c in range(NCORES):
        y[c * BLOC : (c + 1) * BLOC] = (
            results[c]["y"]
            .astype(np.float32)
            .reshape(BLOC, L, C)[:, : L - 1, :]
        )
    return y


def kernel(x: np.ndarray) -> np.ndarray:
    from concourse.bass_utils import run_bass_kernel_spmd

    nc = build_bass()
    x = np.asarray(x, dtype=np.float32)
    res = run_bass_kernel_spmd(nc, _shard_inputs(x), list(range(NCORES)))
    return _gather_outputs(res.results)
